# revision 24
# baseline (speedup 1.0000x reference)
"""DoubleMaskedChamferDistance Trainium2 kernel.

Full inputs: video_feat [128,512,512] f32, lang_feat [128,64,512] f32,
mask_v [128,512] f32, mask_l [128,64] f32  ->  out [128] f32.

Sharding: data-parallel over batch B=128 across 8 cores (16 per core).

Math notes:
 - pd[v,l] = |v|^2 - 2 v.l + |l|^2 ; masked = pd + (1 - mask_v mask_l) * max(pd).
   Since pd >= 0 and max(pd) <= ~1400 on this data, any constant M >= max(pd)
   yields identical axis-mins.  We use M = 32768 and the decomposition
   M*(1-ml[l]) + M*(1-mv[v])  (>= M wherever the pair is invalid, 0 where
   valid), which splits into a per-partition bias (l side) plus a rank-1
   broadcast row (v side) and removes the cross-batch max dependency.
 - Per batch, one PSUM accumulation in [l, v] layout:
       psum[l,v] = -2*ab[l,v]          (4 bf16 matmuls over 128-deep d-chunks)
                 + 1 * a[v]            (2 rank-1 bf16 matmuls over sq partials)
                 + 1 * M*(1-mv)[v]     (1 K=1 rank-1 bf16 matmul)
   and + (b[l] + M*(1-ml[l])) is applied as the ACT bias at evacuation.
 - minsl = min over v: free-dim reduce of the evacuated masked tile.
 - minsv = min over l: PE-transpose masked to [v, l] strips, free-dim reduce.
 - Per-batch partition sums are deferred and reduced once at the end.

Performance-critical structure (vs the first working version):
 - Video is DMA'd with 8KB-contiguous partition lines (v = 4p+s interleave)
   so SWDGE descriptor generation is 4x cheaper and the stream sustains
   ~full HBM rate; all v-indexed rows/columns (mask rows, mask cols) are
   built on-chip in the same interleaved order (ACT view-permute + a tiny
   SBUF->SBUF fold DMA).  Masks ride HWDGE (sync) so they land immediately.
 - DMA issue order is tuned so pair 0 starts early and no later PE idle gap
   exceeds the ~3.4us HAM re-throttle window: lang slice 0, 2 video chunks,
   lang slice 1, 2 chunks, remaining lang, remaining chunks.  Per-DMA SWDGE
   fixed cost (~1-2us) makes finer lang slicing counterproductive.
 - Warm-up PE transposes at t~0 lift the HAM clock gate before real work.
 - Identity affine_selects are the only gpsimd ops ahead of its DMA queue;
   other consts go on vector.  Mask-count reductions + reciprocals run
   under the DMA window, off the final phase's critical path.
 - The video squares are split so chunk 0 follows the DVE evacuation
   immediately instead of the larger ACT copy.

Toolchain constraint honored throughout: every DMA instruction may carry at
most ONE semaphore wait, so DMAs only ever write fresh (never-recycled) tiles
and all data marshalling between tiles is done by compute engines.
"""

import numpy as np

import concourse.bass as bass
import concourse.mybir as mybir
import concourse.tile as tile
from concourse import bacc, masks
from concourse.bass_utils import run_bass_kernel_spmd

N_CORES = 8
B, TV, TL, D = 128, 512, 64, 512
B_LOC = B // N_CORES  # 16
M_CONST = 32768.0
N_WARMUP = 16

F32 = mybir.dt.float32
BF16 = mybir.dt.bfloat16
AX = mybir.AxisListType


def _emit(nc, tc, ctx, video, lang, mask_v, mask_l, out):
    TT = mybir.AluOpType
    AF = mybir.ActivationFunctionType

    consts = ctx.enter_context(tc.tile_pool(name="consts", bufs=1))
    vpool = ctx.enter_context(tc.tile_pool(name="vpool", bufs=1))
    vT = ctx.enter_context(tc.tile_pool(name="vT", bufs=6))
    langp = ctx.enter_context(tc.tile_pool(name="langp", bufs=3))
    sqs = ctx.enter_context(tc.tile_pool(name="sqs", bufs=3))
    smalls = ctx.enter_context(tc.tile_pool(name="smalls", bufs=4))
    maskedp = ctx.enter_context(tc.tile_pool(name="maskedp", bufs=3))
    ps_vT = ctx.enter_context(tc.tile_pool(name="ps_vT", bufs=2, space="PSUM"))
    ps_main = ctx.enter_context(tc.tile_pool(name="ps_main", bufs=2, space="PSUM"))
    ps_small = ctx.enter_context(tc.tile_pool(name="ps_small", bufs=2, space="PSUM"))

    NP = B_LOC // 2  # batch pairs

    # ---- identb first: only its affine_select sits at the head of the gpsimd
    # queue (memset on vector) so the PE warm-ups can start at t~0. ----
    identb = consts.tile([128, 128], BF16)
    nc.vector.memset(identb[:], 0.0)
    masks.make_identity(nc, identb[:], nomemset=True)
    identf = consts.tile([128, 128], F32)
    nc.vector.memset(identf[:], 0.0)

    # ---- small consts on vector (keeps gpsimd free for DMA descriptors) ----
    ones128 = consts.tile([128, 1], F32)
    nc.vector.memset(ones128[:], 1.0)
    ones_bf = consts.tile([1, 64], BF16)
    nc.vector.memset(ones_bf[:], 1.0)
    m_col = consts.tile([128, 1], F32)
    nc.vector.memset(m_col[:], M_CONST)
    ones_mat = consts.tile([128, 64], BF16)
    nc.vector.memset(ones_mat[:], 1.0)
    # half-partition ones vectors to reduce the two halves of paired tiles
    ones_top = consts.tile([128, 1], F32)
    nc.vector.memset(ones_top[:], 0.0)
    nc.vector.memset(ones_top[0:64], 1.0)
    ones_bot = consts.tile([128, 1], F32)
    nc.vector.memset(ones_bot[:], 0.0)
    nc.vector.memset(ones_bot[64:128], 1.0)

    # ---- masks in natural layout via HWDGE (fast start, no Q7 involvement) ----
    maskv_nat = consts.tile([B_LOC, 512], F32)
    nc.sync.dma_start(out=maskv_nat[:], in_=mask_v)
    maskl_pair_nat = consts.tile([NP, 128], F32)
    nc.sync.dma_start(
        out=maskl_pair_nat[:], in_=mask_l.rearrange("(j two) l -> j (two l)", two=2)
    )

    # ---- PE warm-up: dummy transpose-matmuls to lift the HAM clock gate
    # before the first real work arrives. ----
    for w in range(N_WARMUP):
        warm = ps_vT.tile([128, 4, 512], BF16, tag="vt_ps")
        nc.tensor.transpose(warm[:, 0, 0:128], identb[:], identb[:])

    # ---- input DMA stream (SWDGE, casts f32->bf16 in flight).  Pair 0/1's
    # inputs first, then the remaining lang slices (so DMA stays strictly
    # ahead of compute: steady-state pace is 2 video chunks per pair), then
    # the remaining video chunks.  Video partition lines are 8KB-contiguous
    # reads: v = 4p + s. ----
    lang_bf = consts.tile([128, NP, 512], BF16)
    vchunks = []
    for c in range(B_LOC):
        t = vpool.tile([128, 4, 512], BF16, tag=f"vch{c}")
        vchunks.append(t)

    def _load_lang(j0, j1):
        nc.gpsimd.dma_start(
            out=lang_bf[:, j0:j1],
            in_=lang[2 * j0 : 2 * j1].rearrange(
                "(j two) l d -> (two l) j d", two=2
            ),
        )

    def _load_vid(c):
        nc.gpsimd.dma_start(
            out=vchunks[c][:], in_=video[c].rearrange("(p s) d -> p s d", s=4)
        )

    # Order tuned so pair 0 starts ~10us in and no later PE gap exceeds the
    # ~3.4us HAM idle window: pair-0 lang slice, two video chunks, pair-1
    # lang slice, two more chunks, then the remaining lang in one transfer,
    # then the video stream (2 chunks per pair > steady compute pace).
    _load_lang(0, 1)
    _load_vid(0)
    _load_vid(1)
    # identf's affine_select slots in here: ready by the time the mask-column
    # transposes need it, without delaying the first data transfers.
    masks.make_identity(nc, identf[:], nomemset=True)
    _load_lang(1, 2)
    _load_vid(2)
    _load_vid(3)
    _load_lang(2, NP)
    for c in range(4, B_LOC):
        _load_vid(c)

    # ---- mask prep (interleaved v-order: column c = s*128+p  <->  v = 4p+s) --
    # mvperm[b, s, p] = mask_v[b, 4p+s]
    mvperm = maskv_nat[:].rearrange("b (p s) -> b s p", s=4)
    # negv[b, s, p] = M * (1 - mask_v[b, 4p+s])   (exact in bf16: 0 or M)
    negv_bf = consts.tile([B_LOC, 4, 128], BF16)
    nc.scalar.activation(
        negv_bf[:], mvperm, AF.Identity, scale=-M_CONST, bias=m_col[0:B_LOC]
    )
    # fold the 16 per-batch rows onto partition 0 for use as K=1 matmul rhs
    neg_rows = consts.tile([1, B_LOC, 512], BF16)
    nc.sync.dma_start(
        out=neg_rows[:], in_=negv_bf[:].rearrange("b s p -> b (s p)")
    )

    # maskv_cols[p, s, b] = mask_v[b, 4p+s]  (same interleave as psum columns)
    mvc_ps = ps_small.tile([128, 4, B_LOC], F32, tag="ps_sm")
    for s in range(4):
        nc.tensor.transpose(
            mvc_ps[:, s], mvperm[:, s], identf[0:B_LOC, 0:B_LOC]
        )
    maskv_cols = consts.tile([128, 4, B_LOC], F32)
    nc.vector.tensor_copy(maskv_cols[:], mvc_ps[:])

    mlc_ps = ps_small.tile([128, NP], F32, tag="ps_sm")
    nc.tensor.transpose(mlc_ps[:], maskl_pair_nat[:], identf[0:NP, 0:NP])
    # masklT_pair[(two l), j] = mask_l[2 j + two, l]
    masklT_pair = consts.tile([128, NP], F32)
    nc.vector.tensor_copy(masklT_pair[:], mlc_ps[:])
    # mlneg[(two l), j] = M * (1 - mask_l)  -> added to the evacuation bias
    mlneg = consts.tile([128, NP], F32)
    nc.scalar.activation(
        mlneg[:], masklT_pair[:], AF.Identity, scale=-M_CONST, bias=m_col[:]
    )

    # ---- mask-count reductions + reciprocals: data-independent, so they run
    # here (under the DMA window) instead of on the critical path at the end --
    nv_sums = consts.tile([128, B_LOC], F32)
    nc.vector.tensor_reduce(
        nv_sums[:],
        maskv_cols[:].rearrange("p s b -> p b s"),
        axis=AX.X,
        op=TT.add,
    )
    red_nv = ps_small.tile([1, B_LOC], F32, tag="ps_sm")
    nc.tensor.matmul(red_nv[:], ones128[:], nv_sums[:], start=True, stop=True)
    rv = smalls.tile([1, B_LOC], F32, tag="rv")
    nc.vector.reciprocal(rv[:], red_nv[:])
    red_nl_e = ps_small.tile([1, NP], F32, tag="ps_sm")
    nc.tensor.matmul(
        red_nl_e[:], ones_top[:], masklT_pair[:], start=True, stop=True
    )
    rl_e = smalls.tile([1, NP], F32, tag="rl_e")
    nc.vector.reciprocal(rl_e[:], red_nl_e[:])
    red_nl_o = ps_small.tile([1, NP], F32, tag="ps_sm")
    nc.tensor.matmul(
        red_nl_o[:], ones_bot[:], masklT_pair[:], start=True, stop=True
    )
    rl_o = smalls.tile([1, NP], F32, tag="rl_o")
    nc.vector.reciprocal(rl_o[:], red_nl_o[:])

    # collectors (written per pair/batch, reduced once at the end)
    minsv_all = consts.tile([128, B_LOC, 4], BF16)
    minsl_pairs = consts.tile([128, NP], F32)
    b_pairs = consts.tile([128, NP], F32)
    bias_pairs = consts.tile([128, NP], F32)

    for j in range(NP):
        # ---- lang pair work: b, bias (= b + M(1-ml)), langT ----
        sq_l = sqs.tile([128, 512], BF16, tag="sq_l")
        nc.scalar.activation(
            sq_l[:], lang_bf[:, j], AF.Square, accum_out=b_pairs[:, j : j + 1]
        )
        nc.scalar.activation(
            bias_pairs[:, j : j + 1],
            b_pairs[:, j : j + 1],
            AF.Identity,
            bias=mlneg[:, j : j + 1],
        )
        lg_ps = ps_small.tile([128, 4, 128], BF16, tag="ps_sm")
        for k in range(4):
            nc.tensor.transpose(
                lg_ps[:, k], lang_bf[:, j, 128 * k : 128 * (k + 1)], identb[:]
            )
        langT = langp.tile([128, 4, 128], BF16, tag="langT")
        nc.vector.tensor_scalar_mul(langT[:], lg_ps[:], -2.0)

        psum_pair = ps_main.tile([128, 512], F32, tag="psum_T")
        # Two passes over the pair: first both batches' transposes/evacuations/
        # squares, then both batches' matmuls. While batch 0's evacuation runs
        # on DVE/ACT, the PE does batch 1's transposes instead of stalling at
        # the head of its in-order queue on batch 0's matmuls.
        vt_sbs, sq_vTs = [], []
        for t in range(2):
            vstrip = vchunks[2 * j + t]  # [128, 4, 512] bf16: (p, s, d)

            # ---- videoT transposes; evacuations split DVE/ACT ----
            # vt[dd, k, 128s+p] = video[4p+s, 128k+dd]
            vt_sb = vT.tile([128, 4, 512], BF16, tag="vt_sb")
            vt_ps = ps_vT.tile([128, 4, 512], BF16, tag="vt_ps")
            for k in range(4):
                for s in range(4):
                    nc.tensor.transpose(
                        vt_ps[:, k, 128 * s : 128 * (s + 1)],
                        vstrip[:, s, 128 * k : 128 * (k + 1)],
                        identb[:],
                    )
            nc.vector.tensor_copy(vt_sb[:, 0:1], vt_ps[:, 0:1])
            nc.scalar.copy(vt_sb[:, 1:4], vt_ps[:, 1:4])

            # ---- square videoT (split so chunk 0 starts right after the DVE
            # copy, not behind the larger ACT copy); per-v partition sums are
            # broadcast-accumulated by all-ones matmuls ----
            sq_vT = sqs.tile([128, 4, 512], BF16, tag="sq_scr")
            nc.vector.tensor_tensor(
                sq_vT[:, 0:1], vt_sb[:, 0:1], vt_sb[:, 0:1], op=TT.mult
            )
            nc.vector.tensor_tensor(
                sq_vT[:, 1:4], vt_sb[:, 1:4], vt_sb[:, 1:4], op=TT.mult
            )
            sq_sum = sqs.tile([128, 2, 512], BF16, tag="sq_sum")
            nc.vector.tensor_tensor(
                sq_sum[:], sq_vT[:, 0:2], sq_vT[:, 2:4], op=TT.add
            )
            vt_sbs.append(vt_sb)
            sq_vTs.append(sq_sum)

        for t in range(2):
            i = 2 * j + t
            half = psum_pair[64 * t : 64 * (t + 1), :]
            vt_sb, sq_vT = vt_sbs[t], sq_vTs[t]
            for k in range(4):
                nc.tensor.matmul(
                    half,
                    langT[:, k, 64 * t : 64 * (t + 1)],
                    vt_sb[:, k],
                    start=(k == 0),
                    stop=False,
                )
            for k in range(2):
                nc.tensor.matmul(
                    half, ones_mat[:], sq_vT[:, k], start=False, stop=False
                )
            # += M*(1-mv[v]) for every l row (K=1 rank-1)
            nc.tensor.matmul(
                half, ones_bf[:], neg_rows[:, i], start=False, stop=True
            )

        # ---- masked evacuation with +(b + M(1-ml)) bias (bf16), both batches --
        masked_pr = maskedp.tile([128, 512], BF16, tag="masked_pr")
        nc.scalar.activation(
            masked_pr[:],
            psum_pair[:],
            AF.Identity,
            bias=bias_pairs[:, j : j + 1],
            scale=1.0,
        )

        # ---- minsl: min over v (free dim), both batches at once ----
        nc.vector.tensor_reduce(
            minsl_pairs[:, j : j + 1], masked_pr[:], axis=AX.X, op=TT.min
        )

        # ---- minsv: transpose full [128,128] pair-blocks (base 0 only; the
        # hardware rejects transposes with base-64 operands), min over l ----
        o2 = ps_small.tile([128, 4, 2, 64], BF16, tag="ps_sm")
        for s in range(4):
            nc.tensor.transpose(
                o2[:, s],
                masked_pr[:, 128 * s : 128 * (s + 1)],
                identb[:],
            )
        nc.vector.tensor_reduce(
            minsv_all[:, 2 * j : 2 * j + 2, :].rearrange("p t s -> p s t"),
            o2[:],
            axis=AX.X,
            op=TT.min,
        )

    # ---- final: masked sums via ones-matmuls over collected columns.  The
    # mask-count reciprocals (rv, rl_e, rl_o) were computed up front, so the
    # remaining chain is: mask-mult -> reduce -> 3 matmuls -> mults -> add ----
    mv_mask = consts.tile([128, B_LOC, 4], F32)
    nc.vector.tensor_tensor(
        mv_mask[:],
        minsv_all[:],
        maskv_cols[:].rearrange("p s b -> p b s"),
        op=TT.mult,
    )
    mv_sums = consts.tile([128, B_LOC], F32)
    nc.vector.tensor_reduce(mv_sums[:], mv_mask[:], axis=AX.X, op=TT.add)
    mlm = consts.tile([128, NP], F32)
    nc.vector.tensor_tensor(mlm[:], minsl_pairs[:], masklT_pair[:], op=TT.mult)

    red_mv = ps_main.tile([1, B_LOC], F32, tag="psum_T")
    nc.tensor.matmul(red_mv[:], ones128[:], mv_sums[:], start=True, stop=True)
    t1 = smalls.tile([1, B_LOC], F32, tag="t1")
    nc.vector.tensor_tensor(t1[:], red_mv[:], rv[:], op=TT.mult)

    # even/odd batch reductions as separate partition-0 matmuls, written
    # into the interleaved positions of t2 via strided views
    t2 = smalls.tile([1, B_LOC], F32, tag="t2")
    t2v = t2[:].rearrange("a (jj two) -> a jj two", two=2)
    red_ml_e = ps_main.tile([1, NP], F32, tag="psum_T")
    nc.tensor.matmul(red_ml_e[:], ones_top[:], mlm[:], start=True, stop=True)
    nc.vector.tensor_tensor(t2v[:, :, 0], red_ml_e[:], rl_e[:], op=TT.mult)
    red_ml_o = ps_main.tile([1, NP], F32, tag="psum_T")
    nc.tensor.matmul(red_ml_o[:], ones_bot[:], mlm[:], start=True, stop=True)
    nc.vector.tensor_tensor(t2v[:, :, 1], red_ml_o[:], rl_o[:], op=TT.mult)

    out_sb = smalls.tile([1, B_LOC], F32, tag="out_sb")
    nc.vector.tensor_tensor(out_sb[:], t1[:], t2[:], op=TT.add)
    nc.sync.dma_start(out=out[:], in_=out_sb[:])


_CACHED_NC = None


def _get_nc():
    global _CACHED_NC
    if _CACHED_NC is None:
        from contextlib import ExitStack

        nc = bacc.Bacc(
            "TRN2", target_bir_lowering=False, debug=False, num_devices=N_CORES
        )
        video = nc.dram_tensor(
            "video", [B_LOC, TV, D], F32, kind="ExternalInput"
        ).ap()
        lang = nc.dram_tensor("lang", [B_LOC, TL, D], F32, kind="ExternalInput").ap()
        mask_v = nc.dram_tensor(
            "mask_v", [B_LOC, TV], F32, kind="ExternalInput"
        ).ap()
        mask_l = nc.dram_tensor(
            "mask_l", [B_LOC, TL], F32, kind="ExternalInput"
        ).ap()
        out = nc.dram_tensor("out", [1, B_LOC], F32, kind="ExternalOutput").ap()
        with tile.TileContext(nc) as tc:
            with ExitStack() as ctx:
                _emit(nc, tc, ctx, video, lang, mask_v, mask_l, out)
        nc.compile()
        _CACHED_NC = nc
    return _CACHED_NC


def _run(video_feat, lang_feat, mask_v, mask_l, trace=False):
    nc = _get_nc()
    video_feat = np.ascontiguousarray(video_feat, dtype=np.float32)
    lang_feat = np.ascontiguousarray(lang_feat, dtype=np.float32)
    mask_v = np.ascontiguousarray(mask_v, dtype=np.float32)
    mask_l = np.ascontiguousarray(mask_l, dtype=np.float32)
    in_maps = []
    for c in range(N_CORES):
        sl = slice(c * B_LOC, (c + 1) * B_LOC)
        in_maps.append(
            {
                "video": video_feat[sl],
                "lang": lang_feat[sl],
                "mask_v": mask_v[sl],
                "mask_l": mask_l[sl],
            }
        )
    res = run_bass_kernel_spmd(nc, in_maps, list(range(N_CORES)), trace=trace)
    full = np.concatenate(
        [res.results[c]["out"].reshape(-1) for c in range(N_CORES)]
    ).astype(np.float32)
    return full, res


def kernel(video_feat, lang_feat, mask_v, mask_l):
    out, _ = _run(video_feat, lang_feat, mask_v, mask_l, trace=False)
    return out


# revision 25
# speedup vs baseline: 1.0604x; 1.0604x over previous
"""DoubleMaskedChamferDistance Trainium2 kernel.

Full inputs: video_feat [128,512,512] f32, lang_feat [128,64,512] f32,
mask_v [128,512] f32, mask_l [128,64] f32  ->  out [128] f32.

Sharding: data-parallel over batch B=128 across 8 cores (16 per core).

Math notes:
 - pd[v,l] = |v|^2 - 2 v.l + |l|^2 ; masked = pd + (1 - mask_v mask_l) * max(pd).
   Since pd >= 0 and max(pd) <= ~1400 on this data, any constant M >= max(pd)
   yields identical axis-mins.  We use M = 32768 and the decomposition
   M*(1-ml[l]) + M*(1-mv[v])  (>= M wherever the pair is invalid, 0 where
   valid), which splits into a per-partition bias (l side) plus a rank-1
   broadcast row (v side) and removes the cross-batch max dependency.
 - Per batch, one PSUM accumulation in [l, v] layout:
       psum[l,v] = -2*ab[l,v]          (4 bf16 matmuls over 128-deep d-chunks)
                 + 1 * a[v]            (2 rank-1 bf16 matmuls over sq partials)
                 + 1 * M*(1-mv)[v]     (1 K=1 rank-1 bf16 matmul)
   and + (b[l] + M*(1-ml[l])) is applied as the ACT bias at evacuation.
 - minsl = min over v: free-dim reduce of the evacuated masked tile.
 - minsv = min over l: PE-transpose masked to [v, l] strips, free-dim reduce.
 - Per-batch partition sums are deferred and reduced once at the end.

Performance-critical structure (vs the first working version):
 - Video is DMA'd with 8KB-contiguous partition lines (v = 4p+s interleave)
   so SWDGE descriptor generation is 4x cheaper and the stream sustains
   ~full HBM rate; all v-indexed rows/columns (mask rows, mask cols) are
   built on-chip in the same interleaved order (ACT view-permute + a tiny
   SBUF->SBUF fold DMA).  Masks ride HWDGE (sync) so they land immediately.
 - DMA issue order is tuned so pair 0 starts early and no later PE idle gap
   exceeds the ~3.4us HAM re-throttle window: lang slice 0, 2 video chunks,
   lang slice 1, 2 chunks, remaining lang, remaining chunks.  Per-DMA SWDGE
   fixed cost (~1-2us) makes finer lang slicing counterproductive.
 - Warm-up PE transposes at t~0 lift the HAM clock gate before real work.
 - Identity affine_selects are the only gpsimd ops ahead of its DMA queue;
   other consts go on vector.  Mask-count reductions + reciprocals run
   under the DMA window, off the final phase's critical path.
 - The video squares are split so chunk 0 follows the DVE evacuation
   immediately instead of the larger ACT copy.

Toolchain constraint honored throughout: every DMA instruction may carry at
most ONE semaphore wait, so DMAs only ever write fresh (never-recycled) tiles
and all data marshalling between tiles is done by compute engines.
"""

import numpy as np

import concourse.bass as bass
import concourse.mybir as mybir
import concourse.tile as tile
from concourse import bacc, masks
from concourse.bass_utils import run_bass_kernel_spmd

N_CORES = 8
B, TV, TL, D = 128, 512, 64, 512
B_LOC = B // N_CORES  # 16
M_CONST = 32768.0
N_WARMUP = 16

F32 = mybir.dt.float32
BF16 = mybir.dt.bfloat16
AX = mybir.AxisListType


def _emit(nc, tc, ctx, video, lang, mask_v, mask_l, out):
    TT = mybir.AluOpType
    AF = mybir.ActivationFunctionType

    consts = ctx.enter_context(tc.tile_pool(name="consts", bufs=1))
    vpool = ctx.enter_context(tc.tile_pool(name="vpool", bufs=1))
    vT = ctx.enter_context(tc.tile_pool(name="vT", bufs=6))
    langp = ctx.enter_context(tc.tile_pool(name="langp", bufs=3))
    sqs = ctx.enter_context(tc.tile_pool(name="sqs", bufs=3))
    smalls = ctx.enter_context(tc.tile_pool(name="smalls", bufs=4))
    maskedp = ctx.enter_context(tc.tile_pool(name="maskedp", bufs=3))
    ps_vT = ctx.enter_context(tc.tile_pool(name="ps_vT", bufs=2, space="PSUM"))
    ps_main = ctx.enter_context(tc.tile_pool(name="ps_main", bufs=2, space="PSUM"))
    ps_small = ctx.enter_context(tc.tile_pool(name="ps_small", bufs=2, space="PSUM"))

    NP = B_LOC // 2  # batch pairs

    # ---- identb first: only its affine_select sits at the head of the gpsimd
    # queue (memset on vector) so the PE warm-ups can start at t~0. ----
    identb = consts.tile([128, 128], BF16)
    nc.vector.memset(identb[:], 0.0)
    masks.make_identity(nc, identb[:], nomemset=True)
    identf = consts.tile([128, 128], F32)
    nc.vector.memset(identf[:], 0.0)

    # ---- small consts on vector (keeps gpsimd free for DMA descriptors) ----
    ones128 = consts.tile([128, 1], F32)
    nc.vector.memset(ones128[:], 1.0)
    ones_bf = consts.tile([1, 64], BF16)
    nc.vector.memset(ones_bf[:], 1.0)
    m_col = consts.tile([128, 1], F32)
    nc.vector.memset(m_col[:], M_CONST)
    ones_mat = consts.tile([128, 64], BF16)
    nc.vector.memset(ones_mat[:], 1.0)
    # half-partition ones vectors to reduce the two halves of paired tiles
    ones_top = consts.tile([128, 1], F32)
    nc.vector.memset(ones_top[:], 0.0)
    nc.vector.memset(ones_top[0:64], 1.0)
    ones_bot = consts.tile([128, 1], F32)
    nc.vector.memset(ones_bot[:], 0.0)
    nc.vector.memset(ones_bot[64:128], 1.0)

    # ---- masks in natural layout via HWDGE (fast start, no Q7 involvement) ----
    maskv_nat = consts.tile([B_LOC, 512], F32)
    nc.sync.dma_start(out=maskv_nat[:], in_=mask_v)
    maskl_pair_nat = consts.tile([NP, 128], F32)
    nc.sync.dma_start(
        out=maskl_pair_nat[:], in_=mask_l.rearrange("(j two) l -> j (two l)", two=2)
    )

    # ---- PE warm-up: dummy transpose-matmuls to lift the HAM clock gate
    # before the first real work arrives. ----
    for w in range(N_WARMUP):
        warm = ps_vT.tile([128, 4, 512], BF16, tag="vt_ps")
        nc.tensor.transpose(warm[:, 0, 0:128], identb[:], identb[:])

    # ---- input DMA stream (SWDGE, casts f32->bf16 in flight).  Pair 0/1's
    # inputs first, then the remaining lang slices (so DMA stays strictly
    # ahead of compute: steady-state pace is 2 video chunks per pair), then
    # the remaining video chunks.  Video partition lines are 8KB-contiguous
    # reads: v = 4p + s. ----
    lang_bf = consts.tile([128, NP, 512], BF16)
    vchunks = []
    for c in range(B_LOC):
        t = vpool.tile([128, 4, 512], BF16, tag=f"vch{c}")
        vchunks.append(t)

    def _load_lang(j0, j1):
        nc.gpsimd.dma_start(
            out=lang_bf[:, j0:j1],
            in_=lang[2 * j0 : 2 * j1].rearrange(
                "(j two) l d -> (two l) j d", two=2
            ),
        )

    def _load_vid(c):
        nc.gpsimd.dma_start(
            out=vchunks[c][:], in_=video[c].rearrange("(p s) d -> p s d", s=4)
        )

    # Order tuned so pair 0 starts ~10us in and no later PE gap exceeds the
    # ~3.4us HAM idle window: pair-0 lang slice, two video chunks, pair-1
    # lang slice, two more chunks, then the remaining lang in one transfer,
    # then the video stream (2 chunks per pair > steady compute pace).
    _load_lang(0, 1)
    _load_vid(0)
    _load_vid(1)
    # identf's affine_select slots in here: ready by the time the mask-column
    # transposes need it, without delaying the first data transfers.
    masks.make_identity(nc, identf[:], nomemset=True)
    _load_lang(1, 2)
    _load_vid(2)
    _load_vid(3)
    _load_lang(2, NP)
    for c in range(4, B_LOC):
        _load_vid(c)

    # ---- mask prep (interleaved v-order: column c = s*128+p  <->  v = 4p+s) --
    # mvperm[b, s, p] = mask_v[b, 4p+s]
    mvperm = maskv_nat[:].rearrange("b (p s) -> b s p", s=4)
    # negv[b, s, p] = M * (1 - mask_v[b, 4p+s])   (exact in bf16: 0 or M)
    negv_bf = consts.tile([B_LOC, 4, 128], BF16)
    nc.scalar.activation(
        negv_bf[:], mvperm, AF.Identity, scale=-M_CONST, bias=m_col[0:B_LOC]
    )
    # fold the 16 per-batch rows onto partition 0 for use as K=1 matmul rhs
    neg_rows = consts.tile([1, B_LOC, 512], BF16)
    nc.sync.dma_start(
        out=neg_rows[:], in_=negv_bf[:].rearrange("b s p -> b (s p)")
    )

    # maskv_cols[p, s, b] = mask_v[b, 4p+s]  (same interleave as psum columns)
    mvc_ps = ps_small.tile([128, 4, B_LOC], F32, tag="ps_sm")
    for s in range(4):
        nc.tensor.transpose(
            mvc_ps[:, s], mvperm[:, s], identf[0:B_LOC, 0:B_LOC]
        )
    maskv_cols = consts.tile([128, 4, B_LOC], F32)
    nc.vector.tensor_copy(maskv_cols[:], mvc_ps[:])

    mlc_ps = ps_small.tile([128, NP], F32, tag="ps_sm")
    nc.tensor.transpose(mlc_ps[:], maskl_pair_nat[:], identf[0:NP, 0:NP])
    # masklT_pair[(two l), j] = mask_l[2 j + two, l]
    masklT_pair = consts.tile([128, NP], F32)
    nc.vector.tensor_copy(masklT_pair[:], mlc_ps[:])
    # mlneg[(two l), j] = M * (1 - mask_l)  -> added to the evacuation bias
    mlneg = consts.tile([128, NP], F32)
    nc.scalar.activation(
        mlneg[:], masklT_pair[:], AF.Identity, scale=-M_CONST, bias=m_col[:]
    )

    # ---- mask-count reductions + reciprocals: data-independent, so they run
    # here (under the DMA window) instead of on the critical path at the end --
    nv_sums = consts.tile([128, B_LOC], F32)
    nc.vector.tensor_reduce(
        nv_sums[:],
        maskv_cols[:].rearrange("p s b -> p b s"),
        axis=AX.X,
        op=TT.add,
    )
    red_nv = ps_small.tile([1, B_LOC], F32, tag="ps_sm")
    nc.tensor.matmul(red_nv[:], ones128[:], nv_sums[:], start=True, stop=True)
    rv = smalls.tile([1, B_LOC], F32, tag="rv")
    nc.vector.reciprocal(rv[:], red_nv[:])
    red_nl_e = ps_small.tile([1, NP], F32, tag="ps_sm")
    nc.tensor.matmul(
        red_nl_e[:], ones_top[:], masklT_pair[:], start=True, stop=True
    )
    rl_e = smalls.tile([1, NP], F32, tag="rl_e")
    nc.vector.reciprocal(rl_e[:], red_nl_e[:])
    red_nl_o = ps_small.tile([1, NP], F32, tag="ps_sm")
    nc.tensor.matmul(
        red_nl_o[:], ones_bot[:], masklT_pair[:], start=True, stop=True
    )
    rl_o = smalls.tile([1, NP], F32, tag="rl_o")
    nc.vector.reciprocal(rl_o[:], red_nl_o[:])

    # collectors (written per pair/batch, reduced once at the end)
    minsv_all = consts.tile([128, B_LOC, 4], BF16)
    minsl_pairs = consts.tile([128, NP], F32)
    b_pairs = consts.tile([128, NP], F32)
    bias_pairs = consts.tile([128, NP], F32)

    for j in range(NP):
        # ---- lang pair work: b, bias (= b + M(1-ml)), langT ----
        sq_l = sqs.tile([128, 512], BF16, tag="sq_l")
        nc.scalar.activation(
            sq_l[:], lang_bf[:, j], AF.Square, accum_out=b_pairs[:, j : j + 1]
        )
        nc.scalar.activation(
            bias_pairs[:, j : j + 1],
            b_pairs[:, j : j + 1],
            AF.Identity,
            bias=mlneg[:, j : j + 1],
        )
        lg_ps = ps_small.tile([128, 4, 128], BF16, tag="ps_sm")
        for k in range(4):
            nc.tensor.transpose(
                lg_ps[:, k], lang_bf[:, j, 128 * k : 128 * (k + 1)], identb[:]
            )
        # -2x scale-evacuation on ACT: DVE is the steady-state pacer engine
        langT = langp.tile([128, 4, 128], BF16, tag="langT")
        nc.scalar.activation(langT[:], lg_ps[:], AF.Copy, scale=-2.0)

        psum_pair = ps_main.tile([128, 512], F32, tag="psum_T")
        # Two passes over the pair: first both batches' transposes/evacuations/
        # squares, then both batches' matmuls. While batch 0's evacuation runs
        # on DVE/ACT, the PE does batch 1's transposes instead of stalling at
        # the head of its in-order queue on batch 0's matmuls.
        vt_sbs, sq_vTs = [], []
        for t in range(2):
            vstrip = vchunks[2 * j + t]  # [128, 4, 512] bf16: (p, s, d)

            # ---- videoT transposes; evacuations split DVE/ACT ----
            # vt[dd, k, 128s+p] = video[4p+s, 128k+dd]
            vt_sb = vT.tile([128, 4, 512], BF16, tag="vt_sb")
            vt_ps = ps_vT.tile([128, 4, 512], BF16, tag="vt_ps")
            for k in range(4):
                for s in range(4):
                    nc.tensor.transpose(
                        vt_ps[:, k, 128 * s : 128 * (s + 1)],
                        vstrip[:, s, 128 * k : 128 * (k + 1)],
                        identb[:],
                    )
            nc.vector.tensor_copy(vt_sb[:, 0:1], vt_ps[:, 0:1])
            nc.scalar.copy(vt_sb[:, 1:4], vt_ps[:, 1:4])

            # ---- square videoT (split so chunk 0 starts right after the DVE
            # copy, not behind the larger ACT copy); per-v partition sums are
            # broadcast-accumulated by all-ones matmuls ----
            sq_vT = sqs.tile([128, 4, 512], BF16, tag="sq_scr")
            nc.vector.tensor_tensor(
                sq_vT[:, 0:1], vt_sb[:, 0:1], vt_sb[:, 0:1], op=TT.mult
            )
            nc.vector.tensor_tensor(
                sq_vT[:, 1:4], vt_sb[:, 1:4], vt_sb[:, 1:4], op=TT.mult
            )
            sq_sum = sqs.tile([128, 2, 512], BF16, tag="sq_sum")
            nc.vector.tensor_tensor(
                sq_sum[:], sq_vT[:, 0:2], sq_vT[:, 2:4], op=TT.add
            )
            vt_sbs.append(vt_sb)
            sq_vTs.append(sq_sum)

        for t in range(2):
            i = 2 * j + t
            half = psum_pair[64 * t : 64 * (t + 1), :]
            vt_sb, sq_vT = vt_sbs[t], sq_vTs[t]
            for k in range(4):
                nc.tensor.matmul(
                    half,
                    langT[:, k, 64 * t : 64 * (t + 1)],
                    vt_sb[:, k],
                    start=(k == 0),
                    stop=False,
                )
            for k in range(2):
                nc.tensor.matmul(
                    half, ones_mat[:], sq_vT[:, k], start=False, stop=False
                )
            # += M*(1-mv[v]) for every l row (K=1 rank-1)
            nc.tensor.matmul(
                half, ones_bf[:], neg_rows[:, i], start=False, stop=True
            )

        # ---- masked evacuation with +(b + M(1-ml)) bias (bf16), both batches --
        masked_pr = maskedp.tile([128, 512], BF16, tag="masked_pr")
        nc.scalar.activation(
            masked_pr[:],
            psum_pair[:],
            AF.Identity,
            bias=bias_pairs[:, j : j + 1],
            scale=1.0,
        )

        # ---- minsl: min over v (free dim), both batches at once ----
        nc.vector.tensor_reduce(
            minsl_pairs[:, j : j + 1], masked_pr[:], axis=AX.X, op=TT.min
        )

        # ---- minsv: transpose full [128,128] pair-blocks (base 0 only; the
        # hardware rejects transposes with base-64 operands), min over l ----
        o2 = ps_small.tile([128, 4, 2, 64], BF16, tag="ps_sm")
        for s in range(4):
            nc.tensor.transpose(
                o2[:, s],
                masked_pr[:, 128 * s : 128 * (s + 1)],
                identb[:],
            )
        nc.vector.tensor_reduce(
            minsv_all[:, 2 * j : 2 * j + 2, :].rearrange("p t s -> p s t"),
            o2[:],
            axis=AX.X,
            op=TT.min,
        )

    # ---- final: masked sums via ones-matmuls over collected columns.  The
    # mask-count reciprocals (rv, rl_e, rl_o) were computed up front, so the
    # remaining chain is: mask-mult -> reduce -> 3 matmuls -> mults -> add ----
    mv_mask = consts.tile([128, B_LOC, 4], F32)
    nc.vector.tensor_tensor(
        mv_mask[:],
        minsv_all[:],
        maskv_cols[:].rearrange("p s b -> p b s"),
        op=TT.mult,
    )
    mv_sums = consts.tile([128, B_LOC], F32)
    nc.vector.tensor_reduce(mv_sums[:], mv_mask[:], axis=AX.X, op=TT.add)
    mlm = consts.tile([128, NP], F32)
    nc.vector.tensor_tensor(mlm[:], minsl_pairs[:], masklT_pair[:], op=TT.mult)

    red_mv = ps_main.tile([1, B_LOC], F32, tag="psum_T")
    nc.tensor.matmul(red_mv[:], ones128[:], mv_sums[:], start=True, stop=True)
    t1 = smalls.tile([1, B_LOC], F32, tag="t1")
    nc.vector.tensor_tensor(t1[:], red_mv[:], rv[:], op=TT.mult)

    # even/odd batch reductions as separate partition-0 matmuls, written
    # into the interleaved positions of t2 via strided views
    t2 = smalls.tile([1, B_LOC], F32, tag="t2")
    t2v = t2[:].rearrange("a (jj two) -> a jj two", two=2)
    red_ml_e = ps_main.tile([1, NP], F32, tag="psum_T")
    nc.tensor.matmul(red_ml_e[:], ones_top[:], mlm[:], start=True, stop=True)
    nc.vector.tensor_tensor(t2v[:, :, 0], red_ml_e[:], rl_e[:], op=TT.mult)
    red_ml_o = ps_main.tile([1, NP], F32, tag="psum_T")
    nc.tensor.matmul(red_ml_o[:], ones_bot[:], mlm[:], start=True, stop=True)
    nc.vector.tensor_tensor(t2v[:, :, 1], red_ml_o[:], rl_o[:], op=TT.mult)

    out_sb = smalls.tile([1, B_LOC], F32, tag="out_sb")
    nc.vector.tensor_tensor(out_sb[:], t1[:], t2[:], op=TT.add)
    nc.sync.dma_start(out=out[:], in_=out_sb[:])


_CACHED_NC = None


def _get_nc():
    global _CACHED_NC
    if _CACHED_NC is None:
        from contextlib import ExitStack

        nc = bacc.Bacc(
            "TRN2", target_bir_lowering=False, debug=False, num_devices=N_CORES
        )
        video = nc.dram_tensor(
            "video", [B_LOC, TV, D], F32, kind="ExternalInput"
        ).ap()
        lang = nc.dram_tensor("lang", [B_LOC, TL, D], F32, kind="ExternalInput").ap()
        mask_v = nc.dram_tensor(
            "mask_v", [B_LOC, TV], F32, kind="ExternalInput"
        ).ap()
        mask_l = nc.dram_tensor(
            "mask_l", [B_LOC, TL], F32, kind="ExternalInput"
        ).ap()
        out = nc.dram_tensor("out", [1, B_LOC], F32, kind="ExternalOutput").ap()
        with tile.TileContext(nc) as tc:
            with ExitStack() as ctx:
                _emit(nc, tc, ctx, video, lang, mask_v, mask_l, out)
        nc.compile()
        _CACHED_NC = nc
    return _CACHED_NC


def _run(video_feat, lang_feat, mask_v, mask_l, trace=False):
    nc = _get_nc()
    video_feat = np.ascontiguousarray(video_feat, dtype=np.float32)
    lang_feat = np.ascontiguousarray(lang_feat, dtype=np.float32)
    mask_v = np.ascontiguousarray(mask_v, dtype=np.float32)
    mask_l = np.ascontiguousarray(mask_l, dtype=np.float32)
    in_maps = []
    for c in range(N_CORES):
        sl = slice(c * B_LOC, (c + 1) * B_LOC)
        in_maps.append(
            {
                "video": video_feat[sl],
                "lang": lang_feat[sl],
                "mask_v": mask_v[sl],
                "mask_l": mask_l[sl],
            }
        )
    res = run_bass_kernel_spmd(nc, in_maps, list(range(N_CORES)), trace=trace)
    full = np.concatenate(
        [res.results[c]["out"].reshape(-1) for c in range(N_CORES)]
    ).astype(np.float32)
    return full, res


def kernel(video_feat, lang_feat, mask_v, mask_l):
    out, _ = _run(video_feat, lang_feat, mask_v, mask_l, trace=False)
    return out
